# revision 10
# baseline (speedup 1.0000x reference)
"""Trainium2 Bass kernel for nn_LogGD (gnn_message_passing, dense E x E edge attention).

Math (see reference): full dense attention over E=8192 edges with d=256,
row-sharded across 8 NeuronCores (1024 Q-rows per core), K/V replicated.

Decomposition (validated in numpy to 5e-6 rel err fp32 / 2.9e-3 bf16):
  - host: degrees, x0 = x + z_in[in_deg] + z_out[out_deg], gathers xs/xd,
    augmented features fold the edge-weight scale and biases into the matmuls:
      xs_aug = [w*xs, w],  Wq_aug = [Wq | bq]   -> Q0 = xs_aug @ Wq_aug.T
      xd_aug = [xd, 1],    Wk_aug = [Wk | bk]   -> K  = xd_aug @ Wk_aug.T
    b_spatial = Q0@D + K@D computed on host (tiny), C = safe exp shift bound.
  - device (per core, transposed layout so no transposes are ever needed):
      Q0sT [256,1024] = Wq_augT.T-matmuls; KT [256,8192]; Vw [8192,256] row-layout
      for each j-tile (128 cols of K) x i-half (512 Q rows):
        aT = KT_j.T @ Q0sT  (PSUM), expT = exp(aT/16 + (bsp_j - C)) via ACT -> bf16
        zT[f,i]  += Vw_j.T @ expT   (PSUM accumulation over j)
        srow[i]  += ones.T @ expT   (softmax denominators, fused as extra m-tile)
      u = zT * (1/srow) broadcast via ones-matmul; row-sum and row-max over i.
  - host: combine 8 cores' partials, + D terms (softmax rows sum to 1 =>
    a_hat@(V0+D) = a_hat@V0 + D), layernorm + exact-gelu MLP -> [2].
"""

import sys

sys.path.insert(0, "/opt/trn_rl_repo")

import numpy as np
import ml_dtypes

import concourse.bass as bass
from concourse import bacc
import concourse.mybir as mybir
from concourse import tile
from concourse.bass_utils import run_bass_kernel_spmd

N_NODES = 4096
E = 8192
IN_F = 256
D = 256          # out_f
N_CORES = 8
ES = E // N_CORES            # 1024 Q rows per core
CK = 3                       # contraction k-tiles for augmented in_f (257 -> 3*128)
JT = E // 128                # 64 j-tiles
NIH = 2                      # i processed in halves of 512
IHW = ES // NIH              # 512

F32 = mybir.dt.float32
BF16 = mybir.dt.bfloat16

_CACHE = {}


def build_program(trace_ns=False):
    nc = bacc.Bacc(None, target_bir_lowering=False, debug=False)

    wq_d = nc.dram_tensor("wq", [CK, 128, D], BF16, kind="ExternalInput")
    wk_d = nc.dram_tensor("wk", [CK, 128, D], BF16, kind="ExternalInput")
    wv_d = nc.dram_tensor("wv", [CK, 128, D], BF16, kind="ExternalInput")
    xs_d = nc.dram_tensor("xs", [CK, 128, ES], BF16, kind="ExternalInput")
    xd_d = nc.dram_tensor("xd", [CK, 128, E], BF16, kind="ExternalInput")
    bsp_d = nc.dram_tensor("bsp", [128, JT], F32, kind="ExternalInput")
    ew_d = nc.dram_tensor("ew", [128, JT], F32, kind="ExternalInput")
    out_d = nc.dram_tensor("out", [2, 2, 128], F32, kind="ExternalOutput")

    with tile.TileContext(nc) as tc:
        with (
            tc.tile_pool(name="consts", bufs=1) as consts,
            tc.tile_pool(name="et", bufs=4) as etp,
            tc.tile_pool(name="fin", bufs=2) as finp,
            tc.tile_pool(name="pmm", bufs=2, space="PSUM") as pmm,
            tc.tile_pool(name="pacc", bufs=1, space="PSUM") as pacc,
        ):
            wq_sb = consts.tile([128, CK, D], BF16, tag="wq")
            wk_sb = consts.tile([128, CK, D], BF16, tag="wk")
            wv_sb = consts.tile([128, CK, D], BF16, tag="wv")
            xs_sb = consts.tile([128, CK, ES], BF16, tag="xs")
            xd_sb = consts.tile([128, CK, E], BF16, tag="xd")
            bsp_sb = consts.tile([128, JT], F32, tag="bsp")
            ew_sb = consts.tile([128, JT], F32, tag="ew")
            kt_sb = consts.tile([128, 2, E], BF16, tag="kt")
            q_sb = consts.tile([128, 2, ES], BF16, tag="q")
            vw_sb = consts.tile([128, JT, D], BF16, tag="vw")
            ones_j = consts.tile([128, 1], F32, tag="onesj")
            ones_m = consts.tile([1, 128], F32, tag="onesm")
            s_acc = consts.tile([128, ES], F32, tag="sacc")
            fsum = consts.tile([128, 2, NIH], F32, tag="fsum")
            fmax = consts.tile([128, 2, NIH], F32, tag="fmax")

            for k in range(CK):
                nc.sync.dma_start(out=wq_sb[:, k, :], in_=wq_d[k])
                nc.sync.dma_start(out=wk_sb[:, k, :], in_=wk_d[k])
                nc.sync.dma_start(out=wv_sb[:, k, :], in_=wv_d[k])
                nc.sync.dma_start(out=xs_sb[:, k, :], in_=xs_d[k])
                # chunk the big xd load so projections can start per-chunk
                for ch in range(8):
                    csl = slice(ch * (E // 8), (ch + 1) * (E // 8))
                    nc.sync.dma_start(out=xd_sb[:, k, csl], in_=xd_d[k, :, csl])
            nc.sync.dma_start(out=bsp_sb[:], in_=bsp_d[:])
            nc.sync.dma_start(out=ew_sb[:], in_=ew_d[:])
            nc.vector.memset(ones_j[:], 1.0)
            nc.vector.memset(ones_m[:], 1.0)

            # ---- Q0sT [256, 1024] = Wq_aug @ xs_aug^T (bf16, fp32 accum) ----
            for m in range(2):
                for ch in range(ES // 512):
                    pq = pmm.tile([128, 512], F32, tag="mm")
                    for k in range(CK):
                        nc.tensor.matmul(
                            pq[:],
                            wq_sb[:, k, m * 128 : (m + 1) * 128],
                            xs_sb[:, k, ch * 512 : (ch + 1) * 512],
                            start=(k == 0),
                            stop=(k == CK - 1),
                        )
                    nc.vector.tensor_copy(q_sb[:, m, ch * 512 : (ch + 1) * 512], pq[:])

            # ---- KT [256, 8192] = Wk_aug @ xd_aug^T ----
            for m in range(2):
                for ch in range(E // 512):
                    pk = pmm.tile([128, 512], F32, tag="mm")
                    for k in range(CK):
                        nc.tensor.matmul(
                            pk[:],
                            wk_sb[:, k, m * 128 : (m + 1) * 128],
                            xd_sb[:, k, ch * 512 : (ch + 1) * 512],
                            start=(k == 0),
                            stop=(k == CK - 1),
                        )
                    nc.vector.tensor_copy(kt_sb[:, m, ch * 512 : (ch + 1) * 512], pk[:])

            # ---- Vw [8192, 256] row-layout = ew * (xd_aug @ Wv_aug^T) ----
            for j in range(JT):
                pv = pmm.tile([128, D], F32, tag="mm")
                for k in range(CK):
                    nc.tensor.matmul(
                        pv[:],
                        xd_sb[:, k, j * 128 : (j + 1) * 128],
                        wv_sb[:, k, :],
                        start=(k == 0),
                        stop=(k == CK - 1),
                    )
                nc.vector.tensor_scalar_mul(vw_sb[:, j, :], pv[:], ew_sb[:, j : j + 1])

            # ---- main attention loop (both i-halves fused per j-tile) ----
            # zt[m][ihc]: accumulators for z^T [f-tile m, i-chunk ihc]
            zt = [[pacc.tile([128, IHW], F32, tag=f"zt{m}{c}", name=f"zt{m}{c}")
                   for c in range(NIH)] for m in range(2)]
            for j in range(JT):
                jsl = slice(j * 128, (j + 1) * 128)
                pa = pmm.tile([128, ES], F32, tag="mm")
                for dtl in range(2):
                    for ihc in range(NIH):
                        nc.tensor.matmul(
                            pa[:, ihc * IHW : (ihc + 1) * IHW],
                            kt_sb[:, dtl, jsl],
                            q_sb[:, dtl, ihc * IHW : (ihc + 1) * IHW],
                            start=(dtl == 0), stop=(dtl == 1),
                        )
                et = etp.tile([128, ES], BF16, tag="et")
                nc.scalar.activation(
                    et[:],
                    pa[:],
                    mybir.ActivationFunctionType.Exp,
                    bias=bsp_sb[:, j : j + 1],
                    scale=0.0625,
                )
                # softmax denominators on the otherwise-idle vector engine
                if j == 0:
                    nc.vector.tensor_copy(s_acc[:], et[:])
                else:
                    nc.vector.tensor_tensor(
                        s_acc[:], s_acc[:], et[:], mybir.AluOpType.add
                    )
                for m in range(2):
                    for ihc in range(NIH):
                        nc.tensor.matmul(
                            zt[m][ihc][:],
                            vw_sb[:, j, m * 128 : (m + 1) * 128],
                            et[:, ihc * IHW : (ihc + 1) * IHW],
                            start=(j == 0), stop=(j == JT - 1),
                            skip_group_check=True,
                        )

            # ---- normalize + pool ----
            for ihc in range(NIH):
                isl = slice(ihc * IHW, (ihc + 1) * IHW)
                srow = pmm.tile([1, IHW], F32, tag="mm")
                nc.tensor.matmul(srow[:], ones_j[:], s_acc[:, isl])
                rs = finp.tile([1, IHW], F32, tag="rs")
                nc.vector.reciprocal(rs[:], srow[:])
                rb = pmm.tile([128, IHW], F32, tag="mm")
                nc.tensor.matmul(rb[:], ones_m[:], rs[:])
                rbs = finp.tile([128, IHW], F32, tag="rbs")
                nc.vector.tensor_copy(rbs[:], rb[:])
                for m in range(2):
                    u = finp.tile([128, IHW], F32, tag="u")
                    nc.vector.tensor_tensor(
                        u[:], zt[m][ihc][:], rbs[:], mybir.AluOpType.mult
                    )
                    nc.vector.tensor_reduce(
                        fsum[:, m, ihc : ihc + 1], u[:],
                        axis=mybir.AxisListType.X, op=mybir.AluOpType.add,
                    )
                    nc.vector.tensor_reduce(
                        fmax[:, m, ihc : ihc + 1], u[:],
                        axis=mybir.AxisListType.X, op=mybir.AluOpType.max,
                    )

            # ---- combine i-halves, write out ----
            for m in range(2):
                ts_ = finp.tile([128, 1], F32, tag="ts")
                tm_ = finp.tile([128, 1], F32, tag="tm")
                nc.vector.tensor_tensor(
                    ts_[:], fsum[:, m, 0:1], fsum[:, m, 1:2], mybir.AluOpType.add
                )
                nc.vector.tensor_tensor(
                    tm_[:], fmax[:, m, 0:1], fmax[:, m, 1:2], mybir.AluOpType.max
                )
                nc.sync.dma_start(out=out_d[0, m], in_=ts_[:, 0])
                nc.sync.dma_start(out=out_d[1, m], in_=tm_[:, 0])

    nc.finalize()
    return nc


def _host_prep(x, edge_index, edge_weights, Wq_w, Wq_b, Wk_w, Wk_b, Wv_w, Wv_b,
               z_in, z_out):
    src = np.asarray(edge_index[0], dtype=np.int64)
    dst = np.asarray(edge_index[1], dtype=np.int64)
    n = x.shape[0]
    in_deg = np.bincount(src, minlength=n).astype(np.int64)
    out_deg = np.bincount(dst, minlength=n).astype(np.int64)
    x0 = x + z_in[np.clip(in_deg, 0, z_in.shape[0] - 1)] \
           + z_out[np.clip(out_deg, 0, z_out.shape[0] - 1)]
    xs, xd = x0[src], x0[dst]
    w = edge_weights.astype(np.float32)

    xs_aug = np.concatenate([w[:, None] * xs, w[:, None]], axis=1)   # [E, 257]
    xd_aug = np.concatenate([xd, np.ones((E, 1), np.float32)], axis=1)
    Wq_aug = np.concatenate([Wq_w, Wq_b[:, None]], axis=1)           # [256, 257]
    Wk_aug = np.concatenate([Wk_w, Wk_b[:, None]], axis=1)
    Wv_aug = np.concatenate([Wv_w, Wv_b[:, None]], axis=1)
    return xs_aug, xd_aug, Wq_aug, Wk_aug, Wv_aug, w


def _pad_T(a, rows=CK * 128):
    """a [n, c<=rows] -> transposed, zero-padded to [rows, n], as [CK,128,n] bf16."""
    at = np.zeros((rows, a.shape[0]), dtype=np.float32)
    at[: a.shape[1], :] = a.T
    return at.reshape(CK, 128, a.shape[0]).astype(ml_dtypes.bfloat16)


def _gelu(v):
    from scipy.special import erf
    return v * 0.5 * (1.0 + erf(v / np.sqrt(2.0)))


def kernel(**inputs):
    inputs = {k: np.asarray(v) for k, v in inputs.items()}
    (xs_aug, xd_aug, Wq_aug, Wk_aug, Wv_aug, w) = _host_prep(
        inputs["x"], inputs["edge_index"], inputs["edge_weights"],
        inputs["Wq_w"], inputs["Wq_b"], inputs["Wk_w"], inputs["Wk_b"],
        inputs["Wv_w"], inputs["Wv_b"], inputs["z_in"], inputs["z_out"],
    )
    D_ = inputs["D"].astype(np.float32)

    # b_spatial on host (tiny), and a safe shift constant C for exp
    Q0 = xs_aug @ Wq_aug.T
    K = xd_aug @ Wk_aug.T
    bsp = Q0 @ D_ + K @ D_                                            # [E]
    C = float(bsp.max()) + float(
        np.linalg.norm(Q0, axis=1).max() * np.linalg.norm(K, axis=1).max()
    ) / np.sqrt(D)

    wq = _pad_T(Wq_aug)
    wk = _pad_T(Wk_aug)
    wv = _pad_T(Wv_aug)
    xdt = _pad_T(xd_aug)                                              # [CK,128,E]
    xst_full = _pad_T(xs_aug)                                         # [CK,128,E]
    bsp_t = np.ascontiguousarray(
        (bsp - C).reshape(JT, 128).T.astype(np.float32))              # [128, JT]
    ew_t = np.ascontiguousarray(w.reshape(JT, 128).T.astype(np.float32))

    if "nc" not in _CACHE:
        _CACHE["nc"] = build_program()
    nc = _CACHE["nc"]

    in_maps = []
    for c in range(N_CORES):
        in_maps.append({
            "wq": wq, "wk": wk, "wv": wv,
            "xs": np.ascontiguousarray(xst_full[:, :, c * ES : (c + 1) * ES]),
            "xd": xdt, "bsp": bsp_t, "ew": ew_t,
        })

    res = run_bass_kernel_spmd(nc, in_maps, core_ids=list(range(N_CORES)))
    outs = [r["out"] for r in res.results]                            # [2,2,128] each

    S = np.zeros(D, np.float64)
    M = np.full(D, -np.inf)
    for o in outs:
        S += o[0].reshape(D).astype(np.float64)
        M = np.maximum(M, o[1].reshape(D).astype(np.float64))
    S_total = S + E * D_.astype(np.float64)
    M_total = M + D_.astype(np.float64)

    hg = np.concatenate([S_total, M_total])
    mu = hg.mean()
    var = ((hg - mu) ** 2).mean()
    hg = (hg - mu) / np.sqrt(var + 1e-5) * inputs["ln_g"] + inputs["ln_b"]
    h = _gelu(hg @ inputs["fc1_w"].T + inputs["fc1_b"])
    h = _gelu(h @ inputs["fc2_w"].T + inputs["fc2_b"])
    out = h @ inputs["fc3_w"].T + inputs["fc3_b"]
    return out.astype(np.float32)
